# revision 3
# baseline (speedup 1.0000x reference)
"""Trainium kernel for nn_Memory_MultiheadAttention_40132174414424.

Strategy: data-parallel over the fused (B, L) dim — 256 fully independent
attention "slots" split as 32 slots per NeuronCore across 8 cores.  Weights
(512x512) are replicated.  Each core runs the full QKV-projection ->
per-head attention -> output-projection pipeline on its shard; results are
gathered and reshaped back to the full output on the host.
"""

import numpy as np

B, L, E, Q, K, H, NH = 4, 64, 8, 16, 32, 512, 8
D = H // NH          # 64 head dim
S = B * L            # 256 slots
NCORES = 8
SPC = S // NCORES    # 32 slots per core

_COMPILED = {}


def _get_compiled():
    if "fn" in _COMPILED:
        return _COMPILED["fn"]
    import jax
    import jax.numpy as jnp

    scale = np.float32(1.0 / np.sqrt(D))

    def shard_fn(xq, xk, xv, bias, Wq, bq, Wk, bk, Wv, bv, Wo, bo):
        # xq [SPC,E,Q,H]  xk/xv [SPC,E,K,H]  bias [SPC,E,Q,K]
        q = (xq @ (Wq * scale) + bq * scale).reshape(SPC, E, Q, NH, D)
        k = (xk @ Wk + bk).reshape(SPC, E, K, NH, D)
        v = (xv @ Wv + bv).reshape(SPC, E, K, NH, D)
        # head-split without transposes: keep [S,E,W,NH,D] layouts
        logits = jnp.einsum("seqnd,seknd->senqk", q, k) + bias[:, :, None, :, :]
        attn = jax.nn.softmax(logits, axis=-1)
        x = jnp.einsum("senqk,seknd->seqnd", attn, v)
        # faithful torch raw view: [S,E,NH,Q,K] -> [S,Q,NH,K,E], take nh=0
        top = attn.reshape(SPC, Q, NH, K, E)[:, :, 0, :, :]
        out = x.reshape(SPC, E, Q, NH * D) @ Wo + bo
        return out, top

    devs = jax.devices()[:NCORES]
    fn = jax.pmap(shard_fn, devices=devs,
                  in_axes=(0, 0, 0, 0, None, None, None, None, None, None, None, None))
    _COMPILED["fn"] = fn
    return fn


def kernel(query, key_ant, value_ant, bias, Wq, bq, Wk, bk, Wv, bv, Wo, bo,
           num_heads=NH):
    fn = _get_compiled()
    f32 = np.float32
    xq = np.ascontiguousarray(query, f32).reshape(NCORES, SPC, E, Q, H)
    xk = np.ascontiguousarray(key_ant, f32).reshape(NCORES, SPC, E, K, H)
    xv = np.ascontiguousarray(value_ant, f32).reshape(NCORES, SPC, E, K, H)
    bb = np.ascontiguousarray(bias, f32).reshape(NCORES, SPC, E, Q, K)
    out, top = fn(xq, xk, xv, bb,
                  np.asarray(Wq, f32), np.asarray(bq, f32),
                  np.asarray(Wk, f32), np.asarray(bk, f32),
                  np.asarray(Wv, f32), np.asarray(bv, f32),
                  np.asarray(Wo, f32), np.asarray(bo, f32))
    out = np.asarray(out, f32).reshape(B, L, E, Q, H)
    top = np.asarray(top, f32).reshape(B, L, Q, K, E)
    return out, top


# revision 4
# speedup vs baseline: 1.2386x; 1.2386x over previous
"""Trainium kernel for nn_Memory_MultiheadAttention_40132174414424.

Strategy: data-parallel over the fused (B, L) dim — 256 fully independent
attention "slots" split as 32 slots per NeuronCore across 8 cores.  Weights
(512x512) are replicated.  Each core runs the full QKV-projection ->
per-head attention -> output-projection pipeline on its shard; results are
gathered and reshaped back to the full output on the host.
"""

import numpy as np

B, L, E, Q, K, H, NH = 4, 64, 8, 16, 32, 512, 8
D = H // NH          # 64 head dim
S = B * L            # 256 slots
NCORES = 8
SPC = S // NCORES    # 32 slots per core

_COMPILED = {}


def _get_compiled():
    if "fn" in _COMPILED:
        return _COMPILED["fn"]
    import jax
    import jax.numpy as jnp

    scale = np.float32(1.0 / np.sqrt(D))

    def shard_fn(xq, xk, xv, bias, Wq, bq, Wk, bk, Wv, bv, Wo, bo):
        # xq [SPC,E,Q,H]  xk/xv [SPC,E,K,H]  bias [SPC,E,Q,K]
        bf = jnp.bfloat16
        f32 = jnp.float32
        mm = lambda a, w: jax.lax.dot_general(
            a.astype(bf), w.astype(bf), (((a.ndim - 1,), (0,)), ((), ())),
            preferred_element_type=f32)
        q = (mm(xq, Wq * scale) + bq * scale).reshape(SPC, E, Q, NH, D)
        k = (mm(xk, Wk) + bk).reshape(SPC, E, K, NH, D)
        v = (mm(xv, Wv) + bv).reshape(SPC, E, K, NH, D)
        # head-split without transposes: keep [S,E,W,NH,D] layouts
        logits = jnp.einsum("seqnd,seknd->senqk", q.astype(bf), k.astype(bf),
                            preferred_element_type=f32) + bias[:, :, None, :, :]
        attn = jax.nn.softmax(logits, axis=-1)
        x = jnp.einsum("senqk,seknd->seqnd", attn.astype(bf), v.astype(bf),
                       preferred_element_type=f32)
        # faithful torch raw view: [S,E,NH,Q,K] -> [S,Q,NH,K,E], take nh=0
        top = attn.reshape(SPC, Q, NH, K, E)[:, :, 0, :, :]
        out = mm(x.reshape(SPC, E, Q, NH * D), Wo) + bo
        return out, top

    devs = jax.devices()[:NCORES]
    fn = jax.pmap(shard_fn, devices=devs,
                  in_axes=(0, 0, 0, 0, None, None, None, None, None, None, None, None))
    _COMPILED["fn"] = fn
    return fn


def kernel(query, key_ant, value_ant, bias, Wq, bq, Wk, bk, Wv, bv, Wo, bo,
           num_heads=NH):
    fn = _get_compiled()
    f32 = np.float32
    xq = np.ascontiguousarray(query, f32).reshape(NCORES, SPC, E, Q, H)
    xk = np.ascontiguousarray(key_ant, f32).reshape(NCORES, SPC, E, K, H)
    xv = np.ascontiguousarray(value_ant, f32).reshape(NCORES, SPC, E, K, H)
    bb = np.ascontiguousarray(bias, f32).reshape(NCORES, SPC, E, Q, K)
    out, top = fn(xq, xk, xv, bb,
                  np.asarray(Wq, f32), np.asarray(bq, f32),
                  np.asarray(Wk, f32), np.asarray(bk, f32),
                  np.asarray(Wv, f32), np.asarray(bv, f32),
                  np.asarray(Wo, f32), np.asarray(bo, f32))
    out = np.asarray(out, f32).reshape(B, L, E, Q, H)
    top = np.asarray(top, f32).reshape(B, L, Q, K, E)
    return out, top


# revision 5
# speedup vs baseline: 649.2266x; 524.1461x over previous
"""Trainium kernel for nn_Memory_MultiheadAttention_40132174414424.

Strategy: data-parallel over the fused (B, L) dim — 256 fully independent
attention "slots" split as 32 slots per NeuronCore across 8 cores.  Weights
(512x512) are replicated.  Each core runs the full QKV-projection ->
per-head attention -> output-projection pipeline on its shard; results are
gathered and reshaped back to the full output on the host.
"""

import numpy as np

B, L, E, Q, K, H, NH = 4, 64, 8, 16, 32, 512, 8
D = H // NH          # 64 head dim
S = B * L            # 256 slots
NCORES = 8
SPC = S // NCORES    # 32 slots per core

_COMPILED = {}


def _get_compiled():
    if "fn" in _COMPILED:
        return _COMPILED["fn"]
    import jax
    import jax.numpy as jnp

    scale = np.float32(1.0 / np.sqrt(D))

    def shard_fn(xq, xk, xv, bias, Wq, bq, Wk, bk, Wv, bv, Wo, bo):
        # xq [SPC,E,Q,H]  xk/xv [SPC,E,K,H]  bias [SPC,E,Q,K]
        bf = jnp.bfloat16
        f32 = jnp.float32
        mm = lambda a, w: jax.lax.dot_general(
            a.astype(bf), w.astype(bf), (((a.ndim - 1,), (0,)), ((), ())),
            preferred_element_type=f32)
        q = (mm(xq, Wq * scale) + bq * scale).reshape(SPC, E, Q, NH, D)
        k = (mm(xk, Wk) + bk).reshape(SPC, E, K, NH, D)
        v = (mm(xv, Wv) + bv).reshape(SPC, E, K, NH, D)
        # head-split without transposes: keep [S,E,W,NH,D] layouts
        logits = jnp.einsum("seqnd,seknd->senqk", q.astype(bf), k.astype(bf),
                            preferred_element_type=f32) + bias[:, :, None, :, :]
        attn = jax.nn.softmax(logits, axis=-1)
        x = jnp.einsum("senqk,seknd->seqnd", attn.astype(bf), v.astype(bf),
                       preferred_element_type=f32)
        # faithful torch raw view: [S,E,NH,Q,K] -> [S,Q,NH,K,E], take nh=0
        top = attn.reshape(SPC, Q, NH, K, E)[:, :, 0, :, :]
        out = mm(x.reshape(SPC, E, Q, NH * D), Wo) + bo
        return out, top

    devs = jax.devices()[:NCORES]
    fn = jax.pmap(shard_fn, devices=devs,
                  in_axes=(0, 0, 0, 0, None, None, None, None, None, None, None, None))
    _COMPILED["fn"] = fn
    _COMPILED["shard_fn"] = shard_fn
    return fn


def kernel(query, key_ant, value_ant, bias, Wq, bq, Wk, bk, Wv, bv, Wo, bo,
           num_heads=NH):
    fn = _get_compiled()
    f32 = np.float32
    xq = np.ascontiguousarray(query, f32).reshape(NCORES, SPC, E, Q, H)
    xk = np.ascontiguousarray(key_ant, f32).reshape(NCORES, SPC, E, K, H)
    xv = np.ascontiguousarray(value_ant, f32).reshape(NCORES, SPC, E, K, H)
    bb = np.ascontiguousarray(bias, f32).reshape(NCORES, SPC, E, Q, K)
    out, top = fn(xq, xk, xv, bb,
                  np.asarray(Wq, f32), np.asarray(bq, f32),
                  np.asarray(Wk, f32), np.asarray(bk, f32),
                  np.asarray(Wv, f32), np.asarray(bv, f32),
                  np.asarray(Wo, f32), np.asarray(bo, f32))
    out = np.asarray(out, f32).reshape(B, L, E, Q, H)
    top = np.asarray(top, f32).reshape(B, L, Q, K, E)
    return out, top
